# revision 18
# baseline (speedup 1.0000x reference)
"""Trainium2 Bass kernel for nn_CrossWinAttention (window-parallel over 8 cores).

Sharding: 16 attention windows (X*Y) are split 2-per-core. Each core runs the
full pipeline for its 2 windows: LayerNorm -> QKV projections -> RoPE ->
16-head attention (576x576 per head/window) -> softmax -> AV -> head merge ->
mean over n -> output projection -> +skip. Outputs are disjoint slices of the
final tensor, so no cross-core communication is needed.

Device-kernel layout choices (all validated against a numpy mirror):
 - LN gamma/beta folded into Wq/Wk/Wv + biases on host.
 - Per-(head,window) logit scale (head_gate x attn_logit_scale) folded into
   Wq/bq columns on host.
 - Q^T/K^T produced d-major [inner, tok] directly (lhsT = W chunk); V produced
   token-major (lhsT = x^T chunk). x^T comes from PE transposes of LN'd x.
 - RoPE in d-major via a host-side column pair-permutation (rotation partner
   adjacent) so rotate_half becomes a DVE stream_shuffle (32-lane pair swap).
 - Scores computed transposed S^T[k,q] so softmax normalizer comes from an
   all-ones matmul column and P^T feeds AV directly with no transposes.
 - exp(S^T) with no max subtraction (logit range is tiny); denominator
   replicated into 64 psum rows by the ones-matmul, reciprocal on DVE,
   broadcast via gpsimd to the A^T lanes, one multiply normalizes.
 - mean over n folded into Wp (x0.25); bp folded into skip on host.
Matmuls run as float32r (1 cyc/row at free-dim>=256 vs 4 for fp32).
"""
import math
import numpy as np

import concourse.bass as bass
import concourse.bacc as bacc_mod
import concourse.mybir as mybir
import concourse.tile as tile
from concourse import bass_utils
from concourse.alu_op_type import AluOpType
from concourse import library_config

F32 = mybir.dt.float32
F32R = mybir.dt.float32r
AF = mybir.ActivationFunctionType

DIM, HEADS, DH, INNER = 256, 16, 64, 1024
EPS = 1e-5
NCORES, NW, WPC = 8, 16, 2
QN, NTOK = 576, 144
VAUG = 16 * 65
CH5 = [(0, 128), (128, 256), (256, 384), (384, 512), (512, 576)]
CH2 = [(0, 128), (128, 144)]
SWAPMASK = [i ^ 1 for i in range(32)]
USE_F32R = True

_INPUT_SHAPES = {
    'xq': (WPC, QN, DIM), 'xk': (WPC, QN, DIM), 'xv': (WPC, QN, DIM),
    'skipb': (WPC, NTOK, DIM),
    'wq': (128, 2, 2, INNER), 'wk': (128, 2, 2, INNER),
    'wv': (128, 2, VAUG), 'wp': (128, 8, DIM),
    'bq': (128, 2, 8), 'bk': (128, 2, 8),
    'bq_s': (128, 2, 8), 'bk_s': (128, 2, 8),
    'bv': (1, VAUG),
    'cosP': (128, QN), 'sinP': (128, QN),
    'ident': (128, 128),
}


# ---------------------------------------------------------------- host prep
def _host_prep(inputs):
    q = np.asarray(inputs['q'], np.float32)
    k = np.asarray(inputs['k'], np.float32)
    v = np.asarray(inputs['v'], np.float32)
    skip = np.asarray(inputs['skip'], np.float32)
    rope_freqs = np.asarray(inputs['rope_freqs'], np.float32)
    head_gate = np.asarray(inputs['head_gate'], np.float32)
    g_q, b_q = np.asarray(inputs['ln_q_g'], np.float32), np.asarray(inputs['ln_q_b'], np.float32)
    g_k, b_k = np.asarray(inputs['ln_k_g'], np.float32), np.asarray(inputs['ln_k_b'], np.float32)
    g_v, b_v = np.asarray(inputs['ln_v_g'], np.float32), np.asarray(inputs['ln_v_b'], np.float32)
    Wq, bq = np.asarray(inputs['Wq'], np.float32), np.asarray(inputs['bq'], np.float32)
    Wk, bk = np.asarray(inputs['Wk'], np.float32), np.asarray(inputs['bk'], np.float32)
    Wv, bv = np.asarray(inputs['Wv'], np.float32), np.asarray(inputs['bv'], np.float32)
    Wp, bp = np.asarray(inputs['Wp'], np.float32), np.asarray(inputs['bp'], np.float32)
    als = np.asarray(inputs['attn_logit_scale'], np.float32)

    def to_win(t):
        return np.ascontiguousarray(
            t.transpose(0, 2, 3, 1, 4, 5, 6).reshape(NW, QN, DIM))

    qw, kw, vw = to_win(q), to_win(k), to_win(v)
    skipw = skip.reshape(NW, NTOK, DIM)

    s_hl = np.clip(head_gate, 0.0, 1.0)[:, None] * (
        als[None, :] + math.log(DH ** -0.5))

    perm64 = np.empty(64, np.int64)
    perm64[0::2] = np.arange(32)
    perm64[1::2] = np.arange(32) + 32
    permI = np.concatenate([h * 64 + perm64 for h in range(HEADS)])

    Wq1 = g_q[:, None] * Wq
    bq1 = b_q @ Wq + bq
    Wk1 = g_k[:, None] * Wk
    bk1 = b_k @ Wk + bk
    Wv1 = g_v[:, None] * Wv
    bv1 = b_v @ Wv + bv
    # augment V projection with a 65th column per head: w=0, b=1 -> the AV
    # matmul emits the softmax denominator at psum row 64 for free
    Wv_aug = np.zeros((DIM, HEADS * 65), np.float32)
    bv_aug = np.zeros((HEADS * 65,), np.float32)
    for h in range(HEADS):
        Wv_aug[:, h * 65:h * 65 + 64] = Wv1[:, h * 64:(h + 1) * 64]
        bv_aug[h * 65:h * 65 + 64] = bv1[h * 64:(h + 1) * 64]
        bv_aug[h * 65 + 64] = 1.0

    s_col = np.repeat(s_hl, DH, axis=0)          # [INNER, L]
    Wq_l = Wq1[:, :, None] * s_col[None, :, :]   # [DIM, INNER, L]
    bq_l = bq1[:, None] * s_col
    Wq_l = Wq_l[:, permI, :]
    bq_l = bq_l[permI, :]
    Wk1p = Wk1[:, permI]
    bk1p = bk1[permI]

    e = np.arange(128) % 64
    dmap = np.where(e % 2 == 0, e // 2, 32 + e // 2)
    sign = np.where(e % 2 == 0, -1.0, 1.0).astype(np.float32)
    fre = rope_freqs[:QN, :]
    cosP = np.cos(fre[:, dmap]).T.astype(np.float32)
    sinP = (sign[:, None] * np.sin(fre[:, dmap]).T).astype(np.float32)

    Wp_eff = (Wp * 0.25).astype(np.float32)
    skipb = (skipw + bp[None, None, :]).astype(np.float32)
    swap = (np.arange(128) // 32) * 32 + ((np.arange(128) % 32) ^ 1)
    ident = np.eye(128, dtype=np.float32)

    cores = []
    for c in range(NCORES):
        wl = [2 * c, 2 * c + 1]
        bq_c = np.ascontiguousarray(bq_l[:, wl].reshape(8, 128, 2).transpose(1, 2, 0))
        bk_c = np.ascontiguousarray(
            np.broadcast_to(bk1p[:, None], (INNER, 2)).reshape(8, 128, 2).transpose(1, 2, 0))
        core = {
            'xq': qw[wl], 'xk': kw[wl], 'xv': vw[wl],
            'skipb': skipb[wl],
            'wq': np.ascontiguousarray(
                Wq_l[:, :, wl].reshape(2, 128, INNER, 2).transpose(1, 0, 3, 2)),
            'wk': np.ascontiguousarray(
                np.broadcast_to(Wk1p[:, :, None], (DIM, INNER, 2))
                .reshape(2, 128, INNER, 2).transpose(1, 0, 3, 2)),
            'wv': np.ascontiguousarray(Wv_aug.reshape(2, 128, VAUG).transpose(1, 0, 2)),
            'wp': np.ascontiguousarray(Wp_eff.reshape(8, 128, DIM).transpose(1, 0, 2)),
            'bq': bq_c, 'bk': bk_c,
            'bq_s': bq_c[swap], 'bk_s': bk_c[swap],
            'bv': bv_aug.reshape(1, VAUG),
            'cosP': cosP, 'sinP': sinP,
            'ident': ident,
        }
        cores.append({k2: np.ascontiguousarray(v2, dtype=np.float32)
                      for k2, v2 in core.items()})
    return cores


# ------------------------------------------------------------- device kernel
def _mm(ap):
    # operands are already float32r-typed tiles; walrus requires producers to
    # round, so the cast happens at the producing instruction's output dtype.
    return ap


def _emit(tc, nc, d, zout):
    from contextlib import ExitStack
    with ExitStack() as ctx:
        constp = ctx.enter_context(tc.tile_pool(name="const", bufs=1))
        wqkp = ctx.enter_context(tc.tile_pool(name="wqk", bufs=1))
        xrawp = ctx.enter_context(tc.tile_pool(name="xraw", bufs=3))
        xnp = ctx.enter_context(tc.tile_pool(name="xn", bufs=2))
        lnp = ctx.enter_context(tc.tile_pool(name="ln", bufs=2))
        lnkp = ctx.enter_context(tc.tile_pool(name="lnkeep", bufs=1))
        xTp = ctx.enter_context(tc.tile_pool(name="xT", bufs=3))
        qkp = ctx.enter_context(tc.tile_pool(name="qkT", bufs=1))
        vp = ctx.enter_context(tc.tile_pool(name="v", bufs=1))
        ptp = ctx.enter_context(tc.tile_pool(name="PT", bufs=2))
        ropep = ctx.enter_context(tc.tile_pool(name="rope", bufs=2))
        rp = ctx.enter_context(tc.tile_pool(name="rdiv", bufs=1))
        atp = ctx.enter_context(tc.tile_pool(name="apair", bufs=1))
        asump = ctx.enter_context(tc.tile_pool(name="asum", bufs=1))
        smp = ctx.enter_context(tc.tile_pool(name="sm", bufs=2))
        zp = ctx.enter_context(tc.tile_pool(name="z", bufs=2))
        ps_s = ctx.enter_context(tc.tile_pool(name="ps_s", bufs=1, space="PSUM"))
        ps_av = ctx.enter_context(tc.tile_pool(name="ps_av", bufs=2, space="PSUM"))
        # ---- constants
        wv_t = constp.tile([128, 2, VAUG], F32R)
        nc.sync.dma_start(out=wv_t, in_=d['wv'])
        wp_t = constp.tile([128, 8, DIM], F32R)
        nc.sync.dma_start(out=wp_t, in_=d['wp'])
        cos_t = constp.tile([128, QN], F32)
        nc.sync.dma_start(out=cos_t, in_=d['cosP'])
        sin_t = constp.tile([128, QN], F32)
        nc.sync.dma_start(out=sin_t, in_=d['sinP'])
        bias_t = {}
        for nm in ('bq', 'bk', 'bq_s', 'bk_s'):
            bias_t[nm] = constp.tile([128, 2, 8], F32, name=f"bias_{nm}", tag=f"bias_{nm}")
            nc.sync.dma_start(out=bias_t[nm], in_=d[nm])
        bv_t = constp.tile([128, VAUG], F32)
        nc.sync.dma_start(out=bv_t, in_=d['bv'].to_broadcast((128, VAUG)))
        ident = constp.tile([128, 128], F32)
        nc.sync.dma_start(out=ident, in_=d['ident'])

        # ---- LN statistics pre-pass (all 6 tensor-windows) so all ACT Sqrt
        # calls precede all ACT Exp calls: exactly 2 table-set loads.
        lnstat = {}
        for l in range(WPC):
            for nm in ('xq', 'xk', 'xv'):
                stats = lnp.tile([128, 5, 6], F32, tag="stats")
                for tci, (t0, t1) in enumerate(CH5):
                    ts = t1 - t0
                    x = xrawp.tile([128, DIM], F32, tag="xr")
                    nc.sync.dma_start(out=x[0:ts, :], in_=d[nm][l, t0:t1, :])
                    nc.vector.bn_stats(out=stats[0:ts, tci, :], in_=x[0:ts, :])
                aggr = lnp.tile([128, 5, 2], F32, tag="aggr")
                nc.vector.memset(aggr, 1.0)
                for tci, (t0, t1) in enumerate(CH5):
                    ts = t1 - t0
                    nc.vector.bn_aggr(out=aggr[0:ts, tci, :], in_=stats[0:ts, tci, :])
                veps = lnp.tile([128, 5], F32, tag="veps")
                nc.vector.tensor_scalar_add(veps, aggr[:, :, 1], EPS)
                srt = lnp.tile([128, 5], F32, tag="srt")
                nc.scalar.sqrt(out=srt, in_=veps)
                rstd = lnkp.tile([128, 5], F32, tag=f"rstd_{nm}{l}")
                nc.vector.reciprocal(rstd, srt)
                nmr = lnkp.tile([128, 5], F32, tag=f"nmr_{nm}{l}")
                nc.vector.tensor_tensor(out=nmr, in0=aggr[:, :, 0], in1=rstd,
                                        op=AluOpType.mult)
                nc.vector.tensor_scalar_mul(nmr, nmr, -1.0)
                lnstat[(nm, l)] = (rstd, nmr)

        cos2 = cos_t.rearrange("p (a b) -> p a b", a=2)

        for l in range(WPC):
            wq_t = wqkp.tile([128, 2, INNER], F32R, tag="wq")
            nc.sync.dma_start(out=wq_t, in_=d['wq'][:, :, l, :])
            wk_t = wqkp.tile([128, 2, INNER], F32R, tag="wk")
            nc.sync.dma_start(out=wk_t, in_=d['wk'][:, :, l, :])

            # ---- phase A: LN apply + transpose -> x^T per tensor
            xTs = {}
            for nm in ('xq', 'xk', 'xv'):
                rstd, nmr = lnstat[(nm, l)]
                xT = xTp.tile([128, 2, QN], F32R, name=f"xT_{nm}", tag="xT")
                for tci, (t0, t1) in enumerate(CH5):
                    ts = t1 - t0
                    x = xrawp.tile([128, DIM], F32, tag="xr")
                    nc.sync.dma_start(out=x[0:ts, :], in_=d[nm][l, t0:t1, :])
                    xn = xnp.tile([128, DIM], F32, tag="xn")
                    nc.vector.tensor_scalar(
                        out=xn[0:ts, :], in0=x[0:ts, :],
                        scalar1=rstd[0:ts, tci:tci + 1],
                        scalar2=nmr[0:ts, tci:tci + 1],
                        op0=AluOpType.mult, op1=AluOpType.add)
                    ps = ps_s.tile([128, 4, 512], F32, tag="s")
                    nc.tensor.transpose(ps[:, 0, 0:ts], xn[0:ts, 0:128],
                                        ident[0:ts, 0:ts])
                    nc.tensor.transpose(ps[:, 1, 0:ts], xn[0:ts, 128:256],
                                        ident[0:ts, 0:ts])
                    nc.vector.tensor_copy(out=xT[:, 0:2, t0:t1],
                                          in_=ps[:, 0:2, 0:ts])
                xTs[nm] = xT

            # ---- phase A: Q^T / K^T projections + rope
            def proj_qk(xT, w_t, bias, bias_s, tagout):
                oT = qkp.tile([128, 8, QN], F32R, tag=tagout)
                for mc in range(8):
                    ps = ps_av.tile([128, 2, 512], F32, tag="av")
                    for half in range(2):
                        for kc in range(2):
                            nc.tensor.matmul(
                                ps[:, half, 0:288],
                                _mm(w_t[:, kc, mc * 128:(mc + 1) * 128]),
                                _mm(xT[:, kc, half * 288:half * 288 + 288]),
                                start=(kc == 0), stop=(kc == 1))
                    pview = ps[:, 0:2, 0:288]
                    ov = oT[:, mc, :]
                    ov2 = ov.rearrange("p (a b) -> p a b", a=2)
                    nc.vector.scalar_tensor_tensor(
                        out=ov2, in0=pview, scalar=bias[:, l, mc:mc + 1],
                        in1=cos2, op0=AluOpType.add, op1=AluOpType.mult)
                    qsh = ropep.tile([128, QN], F32, tag="qsh")
                    for half in range(2):
                        nc.vector.stream_shuffle(
                            out=qsh[:, half * 288:half * 288 + 288],
                            in_=ps[:, half, 0:288], mask=SWAPMASK)
                    nc.vector.scalar_tensor_tensor(
                        out=qsh, in0=qsh, scalar=bias_s[:, l, mc:mc + 1],
                        in1=sin_t, op0=AluOpType.add, op1=AluOpType.mult)
                    nc.vector.tensor_add(out=ov, in0=ov, in1=qsh)
                return oT

            qT = proj_qk(xTs['xq'], wq_t, bias_t['bq'], bias_t['bq_s'], "qT")
            kT = proj_qk(xTs['xk'], wk_t, bias_t['bk'], bias_t['bk_s'], "kT")

            # ---- phase A: V projection (token-major)
            v_t = vp.tile([128, 5, VAUG], F32R, tag="v")
            for tci, (t0, t1) in enumerate(CH5):
                ts = t1 - t0
                ps = ps_av.tile([128, 2, 512], F32, tag="av")
                ps2 = ps_av.tile([128, 2, 512], F32, tag="av")
                for nh in range(2):
                    for kc in range(2):
                        nc.tensor.matmul(
                            ps[0:ts, nh, 0:512],
                            _mm(xTs['xv'][:, kc, t0:t1]),
                            _mm(wv_t[:, kc, nh * 512:(nh + 1) * 512]),
                            start=(kc == 0), stop=(kc == 1))
                for kc in range(2):
                    nc.tensor.matmul(
                        ps2[0:ts, 0, 0:16],
                        _mm(xTs['xv'][:, kc, t0:t1]),
                        _mm(wv_t[:, kc, 1024:1040]),
                        start=(kc == 0), stop=(kc == 1))
                nc.vector.tensor_tensor(
                    out=v_t[0:ts, tci, 0:1024].rearrange("p (a b) -> p a b", a=2),
                    in0=ps[0:ts, 0:2, 0:512],
                    in1=bv_t[0:ts, 0:1024].rearrange("p (a b) -> p a b", a=2),
                    op=AluOpType.add)
                nc.vector.tensor_tensor(
                    out=v_t[0:ts, tci, 1024:1040],
                    in0=ps2[0:ts, 0, 0:16],
                    in1=bv_t[0:ts, 1024:1040],
                    op=AluOpType.add)

            # ---- phase B: attention, software-pipelined over 8 head pairs
            asum = asump.tile([128, 8, NTOK], F32R, tag="asum")

            def st_pair(pc):
                PT = ptp.tile([128, 5, 1152], F32R, tag="PT")
                for kc, (k0, k1) in enumerate(CH5):
                    ks = k1 - k0
                    ps = ps_s.tile([128, 4, 512], F32, tag="s")
                    for hp in range(2):
                        r0, r1 = hp * 64, hp * 64 + 64
                        for half in range(2):
                            nc.tensor.matmul(
                                ps[0:ks, hp * 2 + half, 0:288],
                                _mm(kT[r0:r1, pc, k0:k1]),
                                _mm(qT[r0:r1, pc, half * 288:half * 288 + 288]),
                                start=True, stop=True,
                                tile_position=(hp * 64, 0))
                    nc.scalar.activation(
                        out=PT[0:ks, kc, :].rearrange("p (a b) -> p a b", a=4),
                        in_=ps[0:ks, 0:4, 0:288], func=AF.Exp)
                return PT

            def av_pair(pc, PT):
                ats = {}
                for hp in range(2):
                    h = pc * 2 + hp
                    ps = ps_av.tile([128, 2, 512], F32, tag="av")
                    for kc, (k0, k1) in enumerate(CH5):
                        ks = k1 - k0
                        for half in range(2):
                            # lhsT = [V_h | ones-col] (65 cols): rows 0:64 of
                            # the psum get A^T, row 64 gets the softmax
                            # denominator -- no extra matmul, dst partition 0.
                            nc.tensor.matmul(
                                ps[0:65, half, 0:288],
                                _mm(v_t[0:ks, kc, h * 65:h * 65 + 65]),
                                _mm(PT[0:ks, kc,
                                       hp * 576 + half * 288:hp * 576 + half * 288 + 288]),
                                start=(kc == 0), stop=(kc == 4))
                    rsb = rp.tile([128, QN], F32, tag="rsb")
                    nc.vector.reciprocal(
                        out=rsb[64:65, :].rearrange("p (a b) -> p a b", a=2),
                        in_=ps[64:65, 0:2, 0:288])
                    bct = rp.tile([128, QN], F32, tag="bct")
                    # replicate the reciprocal row onto the A^T lanes with one
                    # SBUF->SBUF DMA (step-0 middle dim repeats the source row)
                    nc.sync.dma_start(
                        out=bct[0:64, :],
                        in_=rsb[64:65, :].unsqueeze(1)
                        .broadcast_to((1, 64, QN)))
                    at = atp.tile([128, QN], F32, name=f"at{hp}", tag=f"at{hp}")
                    nc.vector.tensor_tensor(
                        out=at[0:64, :].rearrange("p (a b) -> p a b", a=2),
                        in0=ps[0:64, 0:2, 0:288],
                        in1=bct[0:64, :].rearrange("p (a b) -> p a b", a=2),
                        op=AluOpType.mult)
                    ats[hp] = at
                # mean over n: sum the 4 n-groups (x0.25 folded into Wp).
                # Head A sums land at asum rows 0:64 directly; head B's go via
                # a temp then one small DMA to rows 64:128 (DVE cannot cross
                # lanes; DMA can).
                for hp in range(2):
                    at = ats[hp]
                    t1_ = smp.tile([128, NTOK], F32, tag="t1")
                    nc.vector.tensor_add(out=t1_[0:64, :], in0=at[0:64, 0:144],
                                         in1=at[0:64, 144:288])
                    t2_ = smp.tile([128, NTOK], F32, tag="t2")
                    nc.vector.tensor_add(out=t2_[0:64, :], in0=at[0:64, 288:432],
                                         in1=at[0:64, 432:576])
                    if hp == 0:
                        nc.vector.tensor_add(out=asum[0:64, pc, :],
                                             in0=t1_[0:64, :], in1=t2_[0:64, :])
                    else:
                        t3_ = smp.tile([128, NTOK], F32R, tag="t3")
                        nc.vector.tensor_add(out=t3_[0:64, :],
                                             in0=t1_[0:64, :], in1=t2_[0:64, :])
                        nc.sync.dma_start(out=asum[64:128, pc, :],
                                          in_=t3_[0:64, :])
            prev_pt = None
            for pc in range(9):
                cur = st_pair(pc) if pc < 8 else None
                if pc >= 1:
                    av_pair(pc - 1, prev_pt)
                prev_pt = cur

            # ---- final projection + skip, DMA out
            for tc2, (t0, t1) in enumerate(CH2):
                ts = t1 - t0
                ps = ps_av.tile([128, 2, 512], F32, tag="av")
                for c in range(8):
                    nc.tensor.matmul(
                        ps[0:ts, 0, 0:256],
                        _mm(asum[:, c, t0:t1]),
                        _mm(wp_t[:, c, :]),
                        start=(c == 0), stop=(c == 7))
                sk = zp.tile([128, DIM], F32, tag="sk")
                nc.sync.dma_start(out=sk[0:ts, :], in_=d['skipb'][l, t0:t1, :])
                zs = zp.tile([128, DIM], F32, tag="z")
                nc.vector.tensor_tensor(out=zs[0:ts, :], in0=ps[0:ts, 0, 0:256],
                                        in1=sk[0:ts, :], op=AluOpType.add)
                nc.sync.dma_start(out=zout[l, t0:t1, :], in_=zs[0:ts, :])


_F32R_INPUTS = {'wq', 'wk', 'wv', 'wp'}


def build_module():
    # Bacc (not raw Bass): its compile() pass does nop-fusion / wait splitting,
    # without which walrus rejects multi-wait instructions ("Too many sync
    # wait commands").
    nc = bacc_mod.Bacc("TRN2", target_bir_lowering=False, debug=False)
    d = {}
    for name, shape in _INPUT_SHAPES.items():
        dt_ = F32R if name in _F32R_INPUTS else F32
        d[name] = nc.dram_tensor(name, list(shape), dt_, kind="ExternalInput").ap()
    zout = nc.dram_tensor("zout", [WPC, NTOK, DIM], F32, kind="ExternalOutput").ap()
    with tile.TileContext(nc) as tc:
        _emit(tc, nc, d, zout)
    nc.compile()
    return nc


_MODULE = None


def _get_module():
    global _MODULE
    if _MODULE is None:
        _MODULE = build_module()
    return _MODULE


def _gather(zs):
    z = np.stack([w for core_z in zs for w in core_z])
    return np.ascontiguousarray(z.reshape(1, 4, 4, 12, 12, DIM), dtype=np.float32)


def kernel(**inputs):
    cores = _host_prep(inputs)
    nc = _get_module()
    res = bass_utils.run_bass_kernel_spmd(nc, cores, core_ids=list(range(NCORES)))
    zs = [r['zout'] for r in res.results]
    return _gather(zs)


def kernel_traced(**inputs):
    """Like kernel() but requests an NTFF trace; returns (out, exec_time_ns)."""
    cores = _host_prep(inputs)
    nc = _get_module()
    res = bass_utils.run_bass_kernel_spmd(nc, cores, core_ids=list(range(NCORES)),
                                          trace=True)
    zs = [r['zout'] for r in res.results]
    return _gather(zs), res.exec_time_ns


# revision 27
# speedup vs baseline: 389.8452x; 389.8452x over previous
"""Trainium2 Bass kernel for nn_CrossWinAttention (window-parallel over 8 cores).

Sharding: 16 attention windows (X*Y) are split 2-per-core. Each core runs the
full pipeline for its 2 windows: LayerNorm -> QKV projections -> RoPE ->
16-head attention (576x576 per head/window) -> softmax -> AV -> head merge ->
mean over n -> output projection -> +skip. Outputs are disjoint slices of the
final tensor, so no cross-core communication is needed.

Device-kernel layout choices (all validated against a numpy mirror):
 - LN gamma/beta folded into Wq/Wk/Wv + biases on host.
 - Per-(head,window) logit scale (head_gate x attn_logit_scale) folded into
   Wq/bq columns on host.
 - Q^T/K^T produced d-major [inner, tok] directly (lhsT = W chunk); V produced
   token-major (lhsT = x^T chunk). x^T comes from PE transposes of LN'd x.
 - RoPE in d-major via a host-side column pair-permutation (rotation partner
   adjacent) so rotate_half becomes a DVE stream_shuffle (32-lane pair swap).
 - Scores computed transposed S^T[k,q] so softmax normalizer comes from an
   all-ones matmul column and P^T feeds AV directly with no transposes.
 - exp(S^T) with no max subtraction (logit range is tiny); denominator
   replicated into 64 psum rows by the ones-matmul, reciprocal on DVE,
   broadcast via gpsimd to the A^T lanes, one multiply normalizes.
 - mean over n folded into Wp (x0.25); bp folded into skip on host.
Matmuls run as float32r (1 cyc/row at free-dim>=256 vs 4 for fp32).
"""
import math
import numpy as np

import concourse.bass as bass
import concourse.bacc as bacc_mod
import concourse.mybir as mybir
import concourse.tile as tile
from concourse import bass_utils
from concourse.alu_op_type import AluOpType
from concourse import library_config

F32 = mybir.dt.float32
F32R = mybir.dt.float32r
AF = mybir.ActivationFunctionType

DIM, HEADS, DH, INNER = 256, 16, 64, 1024
EPS = 1e-5
NCORES, NW, WPC = 8, 16, 2
QN, NTOK = 576, 144
VAUG = 16 * 65
CH5 = [(0, 128), (128, 256), (256, 384), (384, 512), (512, 576)]
CH2 = [(0, 128), (128, 144)]
SWAPMASK = [i ^ 1 for i in range(32)]
USE_F32R = True

_INPUT_SHAPES = {
    'xq': (WPC, QN, DIM), 'xk': (WPC, QN, DIM), 'xv': (WPC, QN, DIM),
    'skipb': (WPC, NTOK, DIM),
    'wq': (128, 2, 2, INNER), 'wk': (128, 2, 2, INNER),
    'wv': (128, 2, VAUG), 'wp': (128, 8, DIM),
    'bq': (128, 2, 8), 'bk': (128, 2, 8),
    'bq_s': (128, 2, 8), 'bk_s': (128, 2, 8),
    'bv': (1, VAUG),
    'cosP': (128, QN), 'sinP': (128, QN),
    'ident': (128, 128), 'perm128': (128, 128),
}


# ---------------------------------------------------------------- host prep
def _host_prep(inputs):
    q = np.asarray(inputs['q'], np.float32)
    k = np.asarray(inputs['k'], np.float32)
    v = np.asarray(inputs['v'], np.float32)
    skip = np.asarray(inputs['skip'], np.float32)
    rope_freqs = np.asarray(inputs['rope_freqs'], np.float32)
    head_gate = np.asarray(inputs['head_gate'], np.float32)
    g_q, b_q = np.asarray(inputs['ln_q_g'], np.float32), np.asarray(inputs['ln_q_b'], np.float32)
    g_k, b_k = np.asarray(inputs['ln_k_g'], np.float32), np.asarray(inputs['ln_k_b'], np.float32)
    g_v, b_v = np.asarray(inputs['ln_v_g'], np.float32), np.asarray(inputs['ln_v_b'], np.float32)
    Wq, bq = np.asarray(inputs['Wq'], np.float32), np.asarray(inputs['bq'], np.float32)
    Wk, bk = np.asarray(inputs['Wk'], np.float32), np.asarray(inputs['bk'], np.float32)
    Wv, bv = np.asarray(inputs['Wv'], np.float32), np.asarray(inputs['bv'], np.float32)
    Wp, bp = np.asarray(inputs['Wp'], np.float32), np.asarray(inputs['bp'], np.float32)
    als = np.asarray(inputs['attn_logit_scale'], np.float32)

    def to_win(t):
        return np.ascontiguousarray(
            t.transpose(0, 2, 3, 1, 4, 5, 6).reshape(NW, QN, DIM))

    qw, kw, vw = to_win(q), to_win(k), to_win(v)
    skipw = skip.reshape(NW, NTOK, DIM)

    s_hl = np.clip(head_gate, 0.0, 1.0)[:, None] * (
        als[None, :] + math.log(DH ** -0.5))

    perm64 = np.empty(64, np.int64)
    perm64[0::2] = np.arange(32)
    perm64[1::2] = np.arange(32) + 32
    permI = np.concatenate([h * 64 + perm64 for h in range(HEADS)])

    Wq1 = g_q[:, None] * Wq
    bq1 = b_q @ Wq + bq
    Wk1 = g_k[:, None] * Wk
    bk1 = b_k @ Wk + bk
    Wv1 = g_v[:, None] * Wv
    bv1 = b_v @ Wv + bv
    # augment V projection with a 65th column per head: w=0, b=1 -> the AV
    # matmul emits the softmax denominator at psum row 64 for free
    Wv_aug = np.zeros((DIM, HEADS * 65), np.float32)
    bv_aug = np.zeros((HEADS * 65,), np.float32)
    for h in range(HEADS):
        Wv_aug[:, h * 65:h * 65 + 64] = Wv1[:, h * 64:(h + 1) * 64]
        bv_aug[h * 65:h * 65 + 64] = bv1[h * 64:(h + 1) * 64]
        bv_aug[h * 65 + 64] = 1.0

    s_col = np.repeat(s_hl, DH, axis=0)          # [INNER, L]
    Wq_l = Wq1[:, :, None] * s_col[None, :, :]   # [DIM, INNER, L]
    bq_l = bq1[:, None] * s_col
    Wq_l = Wq_l[:, permI, :]
    bq_l = bq_l[permI, :]
    Wk1p = Wk1[:, permI]
    bk1p = bk1[permI]

    e = np.arange(128) % 64
    dmap = np.where(e % 2 == 0, e // 2, 32 + e // 2)
    sign = np.where(e % 2 == 0, -1.0, 1.0).astype(np.float32)
    fre = rope_freqs[:QN, :]
    cosP = np.cos(fre[:, dmap]).T.astype(np.float32)
    sinP = (sign[:, None] * np.sin(fre[:, dmap]).T).astype(np.float32)

    Wp_eff = (Wp * 0.25).astype(np.float32)
    skipb = (skipw + bp[None, None, :]).astype(np.float32)
    swap = (np.arange(128) // 32) * 32 + ((np.arange(128) % 32) ^ 1)
    ident = np.eye(128, dtype=np.float32)
    perm128 = np.eye(128, dtype=np.float32)[:, np.arange(128) ^ 1]

    cores = []
    for c in range(NCORES):
        wl = [2 * c, 2 * c + 1]
        bq_c = np.ascontiguousarray(bq_l[:, wl].reshape(8, 128, 2).transpose(1, 2, 0))
        bk_c = np.ascontiguousarray(
            np.broadcast_to(bk1p[:, None], (INNER, 2)).reshape(8, 128, 2).transpose(1, 2, 0))
        core = {
            'xq': qw[wl], 'xk': kw[wl], 'xv': vw[wl],
            'skipb': skipb[wl],
            'wq': np.ascontiguousarray(
                Wq_l[:, :, wl].reshape(2, 128, INNER, 2).transpose(1, 0, 3, 2)),
            'wk': np.ascontiguousarray(
                np.broadcast_to(Wk1p[:, :, None], (DIM, INNER, 2))
                .reshape(2, 128, INNER, 2).transpose(1, 0, 3, 2)),
            'wv': np.ascontiguousarray(Wv_aug.reshape(2, 128, VAUG).transpose(1, 0, 2)),
            'wp': np.ascontiguousarray(Wp_eff.reshape(8, 128, DIM).transpose(1, 0, 2)),
            'bq': bq_c, 'bk': bk_c,
            'bq_s': bq_c[swap], 'bk_s': bk_c[swap],
            'bv': bv_aug.reshape(1, VAUG),
            'cosP': cosP, 'sinP': sinP,
            'ident': ident, 'perm128': perm128,
        }
        cores.append({k2: np.ascontiguousarray(v2, dtype=np.float32)
                      for k2, v2 in core.items()})
    return cores


# ------------------------------------------------------------- device kernel
def _mm(ap):
    # operands are already float32r-typed tiles; walrus requires producers to
    # round, so the cast happens at the producing instruction's output dtype.
    return ap


def _emit(tc, nc, d, zout):
    from contextlib import ExitStack
    with ExitStack() as ctx:
        constp = ctx.enter_context(tc.tile_pool(name="const", bufs=1))
        wqkp = ctx.enter_context(tc.tile_pool(name="wqk", bufs=1))
        xrawp = ctx.enter_context(tc.tile_pool(name="xraw", bufs=3))
        xnp = ctx.enter_context(tc.tile_pool(name="xn", bufs=2))
        lnp = ctx.enter_context(tc.tile_pool(name="ln", bufs=2))
        lnkp = ctx.enter_context(tc.tile_pool(name="lnkeep", bufs=1))
        xTp = ctx.enter_context(tc.tile_pool(name="xT", bufs=3))
        qkp = ctx.enter_context(tc.tile_pool(name="qkT", bufs=1))
        vp = ctx.enter_context(tc.tile_pool(name="v", bufs=1))
        ptp = ctx.enter_context(tc.tile_pool(name="PT", bufs=2))
        ropep = ctx.enter_context(tc.tile_pool(name="rope", bufs=2))
        rp = ctx.enter_context(tc.tile_pool(name="rdiv", bufs=3))
        atp = ctx.enter_context(tc.tile_pool(name="apair", bufs=2))
        asump = ctx.enter_context(tc.tile_pool(name="asum", bufs=1))
        smp = ctx.enter_context(tc.tile_pool(name="sm", bufs=2))
        zp = ctx.enter_context(tc.tile_pool(name="z", bufs=2))
        ps_s = ctx.enter_context(tc.tile_pool(name="ps_s", bufs=1, space="PSUM"))
        ps_av = ctx.enter_context(tc.tile_pool(name="ps_av", bufs=2, space="PSUM"))
        # ---- LN statistics pre-pass (all 6 tensor-windows) so all ACT Sqrt
        # calls precede all ACT Exp calls: exactly 2 table-set loads.
        lnstat = {}
        for l in range(WPC):
            for nm in ('xq', 'xk', 'xv'):
                stats = lnp.tile([128, 5, 6], F32, tag="stats")
                for tci, (t0, t1) in enumerate(CH5):
                    ts = t1 - t0
                    x = xrawp.tile([128, DIM], F32, tag="xr")
                    nc.sync.dma_start(out=x[0:ts, :], in_=d[nm][l, t0:t1, :])
                    nc.vector.bn_stats(out=stats[0:ts, tci, :], in_=x[0:ts, :])
                aggr = lnp.tile([128, 5, 2], F32, tag="aggr")
                nc.vector.memset(aggr, 1.0)
                for tci, (t0, t1) in enumerate(CH5):
                    ts = t1 - t0
                    nc.vector.bn_aggr(out=aggr[0:ts, tci, :], in_=stats[0:ts, tci, :])
                veps = lnp.tile([128, 5], F32, tag="veps")
                nc.vector.tensor_scalar_add(veps, aggr[:, :, 1], EPS)
                srt = lnp.tile([128, 5], F32, tag="srt")
                nc.scalar.sqrt(out=srt, in_=veps)
                rstd = lnkp.tile([128, 5], F32, tag=f"rstd_{nm}{l}")
                nc.vector.reciprocal(rstd, srt)
                nmr = lnkp.tile([128, 5], F32, tag=f"nmr_{nm}{l}")
                nc.vector.tensor_tensor(out=nmr, in0=aggr[:, :, 0], in1=rstd,
                                        op=AluOpType.mult)
                nc.vector.tensor_scalar_mul(nmr, nmr, -1.0)
                lnstat[(nm, l)] = (rstd, nmr)

        # ---- constants
        wv_t = constp.tile([128, 2, VAUG], F32R)
        nc.sync.dma_start(out=wv_t, in_=d['wv'])
        wp_t = constp.tile([128, 8, DIM], F32R)
        nc.sync.dma_start(out=wp_t, in_=d['wp'])
        cos_t = constp.tile([128, QN], F32)
        nc.sync.dma_start(out=cos_t, in_=d['cosP'])
        sin_t = constp.tile([128, QN], F32)
        nc.sync.dma_start(out=sin_t, in_=d['sinP'])
        bias_t = {}
        for nm in ('bq', 'bk', 'bq_s', 'bk_s'):
            bias_t[nm] = constp.tile([128, 2, 8], F32, name=f"bias_{nm}", tag=f"bias_{nm}")
            nc.sync.dma_start(out=bias_t[nm], in_=d[nm])
        bv_t = constp.tile([128, VAUG], F32)
        nc.sync.dma_start(out=bv_t, in_=d['bv'].to_broadcast((128, VAUG)))
        ident = constp.tile([128, 128], F32)
        nc.sync.dma_start(out=ident, in_=d['ident'])
        perm_t = constp.tile([128, 128], F32R)
        nc.sync.dma_start(out=perm_t, in_=d['perm128'])

        cos2 = cos_t.rearrange("p (a b) -> p a b", a=2)
        sin2 = sin_t.rearrange("p (a b) -> p a b", a=2)

        for l in range(WPC):
            wq_t = wqkp.tile([128, 2, INNER], F32R, tag="wq")
            nc.sync.dma_start(out=wq_t, in_=d['wq'][:, :, l, :])
            wk_t = wqkp.tile([128, 2, INNER], F32R, tag="wk")
            nc.sync.dma_start(out=wk_t, in_=d['wk'][:, :, l, :])

            # ---- phase A: LN apply + transpose -> x^T per tensor
            xTs = {}
            for nm in ('xq', 'xk', 'xv'):
                rstd, nmr = lnstat[(nm, l)]
                xT = xTp.tile([128, 2, QN], F32R, name=f"xT_{nm}", tag="xT")
                for tci, (t0, t1) in enumerate(CH5):
                    ts = t1 - t0
                    x = xrawp.tile([128, DIM], F32, tag="xr")
                    nc.sync.dma_start(out=x[0:ts, :], in_=d[nm][l, t0:t1, :])
                    xn = xnp.tile([128, DIM], F32, tag="xn")
                    nc.vector.tensor_scalar(
                        out=xn[0:ts, :], in0=x[0:ts, :],
                        scalar1=rstd[0:ts, tci:tci + 1],
                        scalar2=nmr[0:ts, tci:tci + 1],
                        op0=AluOpType.mult, op1=AluOpType.add)
                    ps = ps_s.tile([128, 4, 512], F32, tag="s")
                    nc.tensor.transpose(ps[:, 0, 0:ts], xn[0:ts, 0:128],
                                        ident[0:ts, 0:ts])
                    nc.tensor.transpose(ps[:, 1, 0:ts], xn[0:ts, 128:256],
                                        ident[0:ts, 0:ts])
                    nc.scalar.copy(out=xT[:, 0:2, t0:t1],
                                   in_=ps[:, 0:2, 0:ts])
                xTs[nm] = xT

            # ---- phase A: Q^T / K^T projections + rope. The rotate-half
            # partner stream comes from a PE matmul against a constant
            # pair-swap permutation matrix applied to the evacuated raw
            # projection (walrus only allows one free dim on weight APs, so
            # a swapped-column weight read is not an option).
            def proj_qk(xT, w_t, bias, tagout):
                oT = qkp.tile([128, 8, QN], F32R, tag=tagout)
                for mc in range(8):
                    ps = ps_av.tile([128, 2, 512], F32, tag="av")
                    for half in range(2):
                        for kc in range(2):
                            nc.tensor.matmul(
                                ps[:, half, 0:288],
                                _mm(w_t[:, kc, mc * 128:(mc + 1) * 128]),
                                _mm(xT[:, kc, half * 288:half * 288 + 288]),
                                start=(kc == 0), stop=(kc == 1))
                    # evacuate with bias -> raw rope input (f32r for the
                    # permutation matmul)
                    qraw = ropep.tile([128, QN], F32R, tag="qraw")
                    nc.vector.tensor_scalar_add(
                        qraw.rearrange("p (a b) -> p a b", a=2),
                        ps[:, 0:2, 0:288], bias[:, l, mc:mc + 1])
                    pssh = ps_av.tile([128, 2, 512], F32, tag="av")
                    for half in range(2):
                        nc.tensor.matmul(
                            pssh[:, half, 0:288], perm_t,
                            qraw[:, half * 288:half * 288 + 288],
                            start=True, stop=True)
                    ov = oT[:, mc, :]
                    nc.gpsimd.tensor_mul(out=ov, in0=qraw, in1=cos_t)
                    tmp = ropep.tile([128, QN], F32, tag="tmp")
                    nc.vector.tensor_tensor(
                        out=tmp.rearrange("p (a b) -> p a b", a=2),
                        in0=pssh[:, 0:2, 0:288],
                        in1=sin2, op=AluOpType.mult)
                    nc.gpsimd.tensor_add(out=ov, in0=ov, in1=tmp)
                return oT

            qT = proj_qk(xTs['xq'], wq_t, bias_t['bq'], "qT")
            kT = proj_qk(xTs['xk'], wk_t, bias_t['bk'], "kT")

            # ---- phase A: V projection (token-major)
            v_t = vp.tile([128, 5, VAUG], F32R, tag="v")
            for tci, (t0, t1) in enumerate(CH5):
                ts = t1 - t0
                ps = ps_av.tile([128, 2, 512], F32, tag="av")
                ps2 = ps_av.tile([128, 2, 512], F32, tag="av")
                for nh in range(2):
                    for kc in range(2):
                        nc.tensor.matmul(
                            ps[0:ts, nh, 0:512],
                            _mm(xTs['xv'][:, kc, t0:t1]),
                            _mm(wv_t[:, kc, nh * 512:(nh + 1) * 512]),
                            start=(kc == 0), stop=(kc == 1))
                for kc in range(2):
                    nc.tensor.matmul(
                        ps2[0:ts, 0, 0:16],
                        _mm(xTs['xv'][:, kc, t0:t1]),
                        _mm(wv_t[:, kc, 1024:1040]),
                        start=(kc == 0), stop=(kc == 1))
                nc.vector.tensor_tensor(
                    out=v_t[0:ts, tci, 0:1024].rearrange("p (a b) -> p a b", a=2),
                    in0=ps[0:ts, 0:2, 0:512],
                    in1=bv_t[0:ts, 0:1024].rearrange("p (a b) -> p a b", a=2),
                    op=AluOpType.add)
                nc.vector.tensor_tensor(
                    out=v_t[0:ts, tci, 1024:1040],
                    in0=ps2[0:ts, 0, 0:16],
                    in1=bv_t[0:ts, 1024:1040],
                    op=AluOpType.add)

            # ---- phase B: attention, software-pipelined over 8 head pairs
            asum = asump.tile([128, 8, NTOK], F32R, tag="asum")

            def st_av_pair(pc, PTprev):
                """Emit S^T+exp for pair pc, with pair pc-1's AV matmuls
                interleaved per k-chunk so PE has work during each exp."""
                PT = ptp.tile([128, 5, 1152], F32R, name="PT", tag="PT") if pc < 8 else None
                av_ps = {}
                if PTprev is not None:
                    for hp in range(2):
                        av_ps[hp] = ps_av.tile([128, 2, 512], F32, tag="av",
                                               name=f"avps{hp}")
                for kc, (k0, k1) in enumerate(CH5):
                    ks = k1 - k0
                    if PT is not None:
                        ps = ps_s.tile([128, 4, 512], F32, tag="s")
                        for hp in range(2):
                            r0, r1 = hp * 64, hp * 64 + 64
                            for half in range(2):
                                nc.tensor.matmul(
                                    ps[0:ks, hp * 2 + half, 0:288],
                                    _mm(kT[r0:r1, pc, k0:k1]),
                                    _mm(qT[r0:r1, pc, half * 288:half * 288 + 288]),
                                    start=True, stop=True,
                                    tile_position=(hp * 64, 0))
                        nc.scalar.activation(
                            out=PT[0:ks, kc, :].rearrange("p (a b) -> p a b", a=4),
                            in_=ps[0:ks, 0:4, 0:288], func=AF.Exp)
                    if PTprev is not None:
                        for hp in range(2):
                            h = (pc - 1) * 2 + hp
                            for half in range(2):
                                # lhsT = [V_h | ones-col] (65 cols): rows 0:64
                                # get A^T, row 64 the softmax denominator --
                                # no extra matmul, dst partition 0.
                                nc.tensor.matmul(
                                    av_ps[hp][0:65, half, 0:288],
                                    _mm(v_t[0:ks, kc, h * 65:h * 65 + 65]),
                                    _mm(PTprev[0:ks, kc,
                                               hp * 576 + half * 288:hp * 576 + half * 288 + 288]),
                                    start=(kc == 0), stop=(kc == 4))
                if PTprev is not None:
                    av_finish(pc - 1, av_ps)
                return PT

            def av_finish(pc, av_ps):
                ats = {}
                for hp in range(2):
                    ps = av_ps[hp]
                    rsb = rp.tile([128, QN], F32, tag="rsb")
                    nc.vector.reciprocal(
                        out=rsb[64:65, :].rearrange("p (a b) -> p a b", a=2),
                        in_=ps[64:65, 0:2, 0:288])
                    bct = rp.tile([128, QN], F32, tag="bct")
                    # replicate the reciprocal row onto the A^T lanes with one
                    # SBUF->SBUF DMA (step-0 middle dim repeats the source row)
                    nc.sync.dma_start(
                        out=bct[0:64, :],
                        in_=rsb[64:65, :].unsqueeze(1)
                        .broadcast_to((1, 64, QN)))
                    at = atp.tile([128, QN], F32, name=f"at{hp}", tag="at")
                    nc.vector.tensor_tensor(
                        out=at[0:64, :].rearrange("p (a b) -> p a b", a=2),
                        in0=ps[0:64, 0:2, 0:288],
                        in1=bct[0:64, :].rearrange("p (a b) -> p a b", a=2),
                        op=AluOpType.mult)
                    ats[hp] = at
                # mean over n: sum the 4 n-groups (x0.25 folded into Wp).
                # Head A sums land at asum rows 0:64 directly; head B's go via
                # a temp then one small DMA to rows 64:128 (DVE cannot cross
                # lanes; DMA can).
                for hp in range(2):
                    at = ats[hp]
                    t1_ = smp.tile([128, NTOK], F32, tag="t1")
                    nc.gpsimd.tensor_add(out=t1_[0:64, :], in0=at[0:64, 0:144],
                                         in1=at[0:64, 144:288])
                    t2_ = smp.tile([128, NTOK], F32, tag="t2")
                    nc.gpsimd.tensor_add(out=t2_[0:64, :], in0=at[0:64, 288:432],
                                         in1=at[0:64, 432:576])
                    if hp == 0:
                        nc.vector.tensor_add(out=asum[0:64, pc, :],
                                             in0=t1_[0:64, :], in1=t2_[0:64, :])
                    else:
                        t3_ = smp.tile([128, NTOK], F32R, tag="t3")
                        nc.vector.tensor_add(out=t3_[0:64, :],
                                             in0=t1_[0:64, :], in1=t2_[0:64, :])
                        nc.sync.dma_start(out=asum[64:128, pc, :],
                                          in_=t3_[0:64, :])
            prev_pt = None
            for pc in range(9):
                prev_pt = st_av_pair(pc, prev_pt)

            # ---- final projection + skip, DMA out
            for tc2, (t0, t1) in enumerate(CH2):
                ts = t1 - t0
                ps = ps_av.tile([128, 2, 512], F32, tag="av")
                for c in range(8):
                    nc.tensor.matmul(
                        ps[0:ts, 0, 0:256],
                        _mm(asum[:, c, t0:t1]),
                        _mm(wp_t[:, c, :]),
                        start=(c == 0), stop=(c == 7))
                sk = zp.tile([128, DIM], F32, tag="sk")
                nc.sync.dma_start(out=sk[0:ts, :], in_=d['skipb'][l, t0:t1, :])
                zs = zp.tile([128, DIM], F32, tag="z")
                nc.vector.tensor_tensor(out=zs[0:ts, :], in0=ps[0:ts, 0, 0:256],
                                        in1=sk[0:ts, :], op=AluOpType.add)
                nc.sync.dma_start(out=zout[l, t0:t1, :], in_=zs[0:ts, :])


_F32R_INPUTS = {'wq', 'wk', 'wv', 'wp', 'perm128'}


def build_module():
    # Bacc (not raw Bass): its compile() pass does nop-fusion / wait splitting,
    # without which walrus rejects multi-wait instructions ("Too many sync
    # wait commands").
    nc = bacc_mod.Bacc("TRN2", target_bir_lowering=False, debug=False)
    d = {}
    for name, shape in _INPUT_SHAPES.items():
        dt_ = F32R if name in _F32R_INPUTS else F32
        d[name] = nc.dram_tensor(name, list(shape), dt_, kind="ExternalInput").ap()
    zout = nc.dram_tensor("zout", [WPC, NTOK, DIM], F32, kind="ExternalOutput").ap()
    with tile.TileContext(nc) as tc:
        _emit(tc, nc, d, zout)
    nc.compile()
    return nc


_MODULE = None


def _get_module():
    global _MODULE
    if _MODULE is None:
        _MODULE = build_module()
    return _MODULE


def _gather(zs):
    z = np.stack([w for core_z in zs for w in core_z])
    return np.ascontiguousarray(z.reshape(1, 4, 4, 12, 12, DIM), dtype=np.float32)


def kernel(**inputs):
    cores = _host_prep(inputs)
    nc = _get_module()
    res = bass_utils.run_bass_kernel_spmd(nc, cores, core_ids=list(range(NCORES)))
    zs = [r['zout'] for r in res.results]
    return _gather(zs)


def kernel_traced(**inputs):
    """Like kernel() but requests an NTFF trace; returns (out, exec_time_ns)."""
    cores = _host_prep(inputs)
    nc = _get_module()
    res = bass_utils.run_bass_kernel_spmd(nc, cores, core_ids=list(range(NCORES)),
                                          trace=True)
    zs = [r['zout'] for r in res.results]
    return _gather(zs), res.exec_time_ns
